# revision 1
# baseline (speedup 1.0000x reference)
"""Trainium2 Bass kernel for a single attention head (nn_AttentionHead).

Problem: B=16, S=2048, W=768, H=64.
  Q = input @ Wq + bq ; K = input @ Wk + bk ; V = input @ Wv + bv
  scores = Q K^T / sqrt(H), key-padding mask, softmax, out = attn @ V.

Sharding: data-parallel over batch across 8 cores (2 samples per core).

Per-core algorithm (all matmuls bf16, fp32 PSUM accumulation):
  1. TensorE-transpose input tiles (bf16) -> inpT [W, S].
  2. QK^T projection with packed stationary [Wq/8 | Wk] -> Q^T rows 0:64
     (pre-scaled by 1/sqrt(H)), K^T rows 64:128.
  3. V^T projection per sample; V rebuilt natural ([S, H]) via TensorE
     transposes of the stacked [V^T_b0; V^T_b1].
  4. Scores transposed: S^T[key, q] = K^T.T @ Q^T (contract = H = 64).
     PACK>=1: two key tiles run concurrently via tile_position row tiling.
     PACK==0: contract zero-padded to 128 (same cycles, plain array mode).
  5. exp on ScalarE straight out of PSUM with per-partition (= per-key)
     mask bias: exp(s + (-100 if masked else 0)) -> P^T bf16. Softmax
     max-subtraction skipped (scores ~ N(0,1); exp cannot overflow).
  6. O'^T = V'.T @ P^T accumulated over key tiles in PSUM, V' = [V | ones]
     (65 columns). Row 64 = softmax denominator D.
  7. Host epilogue: O = O'[:64] / D, transpose to [B, S, H].
"""

import functools
import os

import numpy as np

import concourse.bass as bass
import concourse.bacc as bacc
import concourse.mybir as mybir
import concourse.tile as tile
from concourse.bass_utils import run_bass_kernel_spmd
from concourse.masks import make_identity

F32 = mybir.dt.float32
BF16 = mybir.dt.bfloat16
I32 = mybir.dt.int32
AF = mybir.ActivationFunctionType
ALU = mybir.AluOpType

P = 128
B_PER_CORE = 2
S = 2048
W = 768
H = 64
NW = W // P      # 6 contraction chunks for the projections
NST = S // P     # 16 sequence tiles
NKT = S // P     # 16 key tiles
NQC = S // 512   # 4 query chunks of 512
N_CORES = 8
MASK_BIAS = -100.0  # additive bias for masked keys; exp(s - 100) == 0 in bf16
QSCALE = 0.125      # 1/sqrt(H)

# 0: no tile_position packing (contract zero-padded to 128)
# 1: row-packed score matmuls (2 key tiles concurrently)
# 2: + col-packed V^T projection (both samples concurrently)
PACK = int(os.environ.get("KERNEL_PACK", "0"))


def _prologue(nc, tc, pools, inp_e, mask_e, w_e, b_e):
    """Everything before the attention loop. Uses its own PSUM pool, which the
    caller closes before opening the attention-phase PSUM pools."""
    (cpool, wstage, io, castp, inpTp, qkp, vtp, vpp, smallp, pro_ps) = pools

    ident = cpool.tile([P, P], BF16, name="ident", tag="ident")
    make_identity(nc, ident)

    wqk = cpool.tile([P, NW, 2 * H], BF16, name="wqk", tag="wqk")
    # V stationary padded to 128 output columns (cols 64:128 zero) so the
    # matmul stays in plain 128x128 array mode when PACK < 2.
    wv = cpool.tile([P, NW, P], BF16, name="wv", tag="wv")
    nc.vector.memset(wv[:, :, H:P], 0.0)
    bias_qk = cpool.tile([P, 1], F32, name="bias_qk", tag="bias_qk")
    bias_v = cpool.tile([P, 1], F32, name="bias_v", tag="bias_v")

    for name, dst, scale in (
        ("Wq", wqk[:, :, 0:H], QSCALE),
        ("Wk", wqk[:, :, H : 2 * H], None),
        ("Wv", wv[:, :, 0:H], None),
    ):
        st = wstage.tile([P, NW, H], F32, name=f"wst_{name}", tag=f"wst_{name}")
        nc.gpsimd.dma_start(out=st, in_=w_e[name].rearrange("(o p) h -> p o h", p=P))
        if scale is not None:
            nc.vector.tensor_scalar_mul(dst, st, scale)
        else:
            nc.vector.tensor_copy(dst, st)

    with nc.allow_non_contiguous_dma(reason="tiny one-time bias loads"):
        nc.gpsimd.dma_start(out=bias_qk[0:H, :], in_=b_e["bq"][:, None])
        nc.gpsimd.dma_start(out=bias_qk[H:P, :], in_=b_e["bk"][:, None])
        nc.gpsimd.dma_start(out=bias_v[0:H, :], in_=b_e["bv"][:, None])
        nc.gpsimd.dma_start(out=bias_v[H:P, :], in_=b_e["bv"][:, None])
    nc.vector.tensor_scalar_mul(bias_qk[0:H, :], bias_qk[0:H, :], QSCALE)

    inpT_l, qk_l, qx_l, kx_l, ebias_l = [], [], [], [], []
    for b in range(B_PER_CORE):
        ebias = smallp.tile([P, NKT], F32, name=f"ebias{b}", tag=f"ebias{b}")
        mask_i = smallp.tile([P, NKT], I32, name=f"mask{b}", tag=f"mask{b}")
        with nc.allow_non_contiguous_dma(reason="mask transposed load (8KB)"):
            nc.gpsimd.dma_start(
                out=mask_i, in_=mask_e[b, 0, :].rearrange("(t p) -> p t", p=P)
            )
        # mask in {0,1} -> bias in {-100, 0}
        nc.vector.tensor_scalar(ebias, mask_i, -MASK_BIAS, MASK_BIAS, ALU.mult, ALU.add)
        ebias_l.append(ebias)

        iT = inpTp.tile([P, NW, S], BF16, name=f"inpT{b}", tag=f"inpT{b}")
        for st_i in range(NST):
            raw = io.tile([P, W], F32, tag="io", name=f"in_{b}_{st_i}")
            nc.sync.dma_start(out=raw, in_=inp_e[b, st_i * P : (st_i + 1) * P, :])
            cst = castp.tile([P, W], BF16, tag="cast", name=f"cast_{b}_{st_i}")
            nc.vector.tensor_copy(cst, raw)
            pT = pro_ps.tile([P, W], BF16, tag="pro", name=f"psT_{b}_{st_i}")
            for wc in range(NW):
                nc.tensor.transpose(
                    pT[:, wc * P : (wc + 1) * P], cst[:, wc * P : (wc + 1) * P], ident
                )
            # Split these evacuations between ScalarE (idle during the
            # prologue) and VectorE (the prologue bottleneck otherwise).
            evac_eng = nc.scalar.copy if True else nc.vector.tensor_copy
            evac_eng(
                iT[:, :, st_i * P : (st_i + 1) * P],
                pT.rearrange("p (o c) -> p o c", c=P),
            )
        inpT_l.append(iT)

        qk_sb = qkp.tile([P, S], BF16, name=f"qk{b}", tag=f"qk{b}")
        for qc in range(NQC):
            ps = pro_ps.tile([P, 512], F32, tag="pro", name=f"psQK_{b}_{qc}")
            for wc in range(NW):
                nc.tensor.matmul(
                    ps,
                    wqk[:, wc, :],
                    iT[:, wc, qc * 512 : (qc + 1) * 512],
                    start=(wc == 0),
                    stop=(wc == NW - 1),
                )
            nc.vector.tensor_scalar(
                qk_sb[:, qc * 512 : (qc + 1) * 512], ps, bias_qk, None, ALU.add
            )
        qk_l.append(qk_sb)

        if PACK >= 1:
            # swapped-halves copy: rows 0:64 = K^T, rows 64:128 = Q^T
            qkx = qkp.tile([P, S], BF16, name=f"qkx{b}", tag=f"qkx{b}")
            nc.sync.dma_start(out=qkx[0:H, :], in_=qk_sb[H:P, :])
            nc.sync.dma_start(out=qkx[H:P, :], in_=qk_sb[0:H, :])
            qx_l.append(qkx)
            kx_l.append(qkx)
        else:
            # zero-padded full-contract copies
            qz = qkp.tile([P, S], BF16, name=f"qz{b}", tag=f"qz{b}")
            kz = qkp.tile([P, S], BF16, name=f"kz{b}", tag=f"kz{b}")
            nc.vector.memset(qz[H:P, :], 0.0)
            nc.vector.memset(kz[H:P, :], 0.0)
            nc.sync.dma_start(out=qz[0:H, :], in_=qk_sb[0:H, :])
            nc.sync.dma_start(out=kz[0:H, :], in_=qk_sb[H:P, :])
            qx_l.append(qz)
            kx_l.append(kz)

    # ---- V^T projection + V' = [V | ones] ----
    vt_sb = vtp.tile([P, S], BF16, name="vt_sb", tag="vt")  # rows 0:64 b0, 64:128 b1
    if PACK >= 2:
        for qc in range(NQC):
            ps_a = pro_ps.tile([P, 512], F32, tag="pro", name=f"psVa_{qc}")
            ps_b = pro_ps.tile([P, 512], F32, tag="pro", name=f"psVb_{qc}")
            for wc in range(NW):
                nc.tensor.matmul(
                    ps_a[0:H, :],
                    wv[:, wc, 0:H],
                    inpT_l[0][:, wc, qc * 512 : (qc + 1) * 512],
                    start=(wc == 0),
                    stop=(wc == NW - 1),
                )
                nc.tensor.matmul(
                    ps_b[H:P, :],
                    wv[:, wc, 0:H],
                    inpT_l[1][:, wc, qc * 512 : (qc + 1) * 512],
                    start=(wc == 0),
                    stop=(wc == NW - 1),
                )
            nc.vector.tensor_scalar(
                vt_sb[0:H, qc * 512 : (qc + 1) * 512], ps_a[0:H, :],
                bias_v[0:H, :], None, ALU.add,
            )
            nc.vector.tensor_scalar(
                vt_sb[H:P, qc * 512 : (qc + 1) * 512], ps_b[H:P, :],
                bias_v[H:P, :], None, ALU.add,
            )
    else:
        vstage = vtp.tile([H, S], BF16, name="vstage", tag="vstage")
        for b in range(B_PER_CORE):
            for qc in range(NQC):
                ps = pro_ps.tile([P, 512], F32, tag="pro", name=f"psV_{b}_{qc}")
                for wc in range(NW):
                    nc.tensor.matmul(
                        ps,
                        wv[:, wc, :],
                        inpT_l[b][:, wc, qc * 512 : (qc + 1) * 512],
                        start=(wc == 0),
                        stop=(wc == NW - 1),
                    )
                dst = (
                    vt_sb[0:H, qc * 512 : (qc + 1) * 512]
                    if b == 0
                    else vstage[:, qc * 512 : (qc + 1) * 512]
                )
                nc.vector.tensor_scalar(dst, ps[0:H, :], bias_v[0:H, :], None, ALU.add)
        nc.sync.dma_start(out=vt_sb[H:P, :], in_=vstage)

    vprime = []
    for b in range(B_PER_CORE):
        vp = vpp.tile([P, NKT, H + 1], BF16, name=f"vp{b}", tag=f"vp{b}")
        nc.vector.memset(vp[:, :, H], 1.0)
        vprime.append(vp)
    for g in range(2):
        psv = pro_ps.tile([P, 8 * P], BF16, tag="pro", name=f"psVt_{g}")
        for j in range(8):
            st_i = g * 8 + j
            nc.tensor.transpose(
                psv[:, j * P : (j + 1) * P], vt_sb[:, st_i * P : (st_i + 1) * P], ident
            )
        pv3 = psv.rearrange("p (j c) -> p j c", c=P)
        nc.vector.tensor_copy(vprime[0][:, g * 8 : (g + 1) * 8, 0:H], pv3[:, :, 0:H])
        nc.vector.tensor_copy(vprime[1][:, g * 8 : (g + 1) * 8, 0:H], pv3[:, :, H:P])

    return qk_l, qx_l, kx_l, ebias_l, vprime


def _build(nc, tc, inp_e, mask_e, w_e, b_e, out_e):
    with (
        tc.tile_pool(name="const", bufs=1) as cpool,
        tc.tile_pool(name="qkp", bufs=1) as qkp,
        tc.tile_pool(name="vpp", bufs=1) as vpp,
        tc.tile_pool(name="ptp", bufs=4) as ptp,
        tc.tile_pool(name="oup", bufs=1) as oup,
        tc.tile_pool(name="smallp", bufs=1) as smallp,
    ):
        with (
            tc.tile_pool(name="wstage", bufs=1) as wstage,
            tc.tile_pool(name="io", bufs=4) as io,
            tc.tile_pool(name="castp", bufs=3) as castp,
            tc.tile_pool(name="inpTp", bufs=1) as inpTp,
            tc.tile_pool(name="vtp", bufs=1) as vtp,
            tc.tile_pool(name="pro_ps", bufs=2, space="PSUM") as pro_ps,
        ):
            pools = (cpool, wstage, io, castp, inpTp, qkp, vtp, vpp, smallp, pro_ps)
            qk_l, qx_l, kx_l, ebias_l, vprime = _prologue(
                nc, tc, pools, inp_e, mask_e, w_e, b_e
            )

        # ---- attention: S^T -> exp -> O'^T (prologue PSUM pool closed) ----
        with (
            tc.tile_pool(name="s_ps", bufs=2, space="PSUM") as s_ps,
            tc.tile_pool(name="o_ps", bufs=1, space="PSUM") as o_ps,
        ):
            for b in range(B_PER_CORE):
                pso = o_ps.tile([P, S], F32, name=f"psO{b}", tag="psO")
                for kt in range(NKT):
                    if PACK >= 1:
                        base = 0 if kt % 2 == 0 else H  # alternate array halves
                        lhs = kx_l[b] if base == 0 else qk_l[b]
                        rhs = qk_l[b] if base == 0 else qx_l[b]
                        lhs_ap = lhs[base : base + H, kt * P : (kt + 1) * P]
                    else:
                        base = 0
                        lhs_ap = kx_l[b][:, kt * P : (kt + 1) * P]
                        rhs = qx_l[b]
                    ptile = ptp.tile([P, S], BF16, tag="pt", name=f"pt_{b}_{kt}")
                    for qh in range(NQC // 2):
                        pss = s_ps.tile(
                            [P, 1024], F32, tag="ps_s", name=f"psS_{b}_{kt}_{qh}"
                        )
                        for qi in range(2):
                            qc = 2 * qh + qi
                            if PACK >= 1:
                                rhs_ap = rhs[base : base + H, qc * 512 : (qc + 1) * 512]
                            else:
                                rhs_ap = rhs[:, qc * 512 : (qc + 1) * 512]
                            nc.tensor.matmul(
                                pss[:, qi * 512 : (qi + 1) * 512],
                                lhs_ap,
                                rhs_ap,
                                start=True,
                                stop=True,
                            )
                        nc.scalar.activation(
                            ptile[:, qh * 1024 : (qh + 1) * 1024],
                            pss,
                            AF.Exp,
                            bias=ebias_l[b][:, kt : kt + 1],
                            scale=1.0,
                        )
                    for qc in range(NQC):
                        nc.tensor.matmul(
                            pso[0 : H + 1, qc * 512 : (qc + 1) * 512],
                            vprime[b][:, kt, :],
                            ptile[:, qc * 512 : (qc + 1) * 512],
                            start=(kt == 0),
                            stop=(kt == NKT - 1),
                        )
                ou = oup.tile([H + 1, S], F32, name=f"ou{b}", tag=f"ou{b}")
                nc.vector.tensor_copy(ou, pso[0 : H + 1, :])
                nc.sync.dma_start(out=out_e[b], in_=ou)


def build_nc() -> bass.Bass:
    nc = bacc.Bacc()
    inp_e = nc.declare_dram_parameter("input", [B_PER_CORE, S, W], F32, isOutput=False)
    mask_e = nc.declare_dram_parameter("mask", [B_PER_CORE, 1, S], I32, isOutput=False)
    w_e = {}
    b_e = {}
    for wname, bname in (("Wq", "bq"), ("Wk", "bk"), ("Wv", "bv")):
        w_e[wname] = nc.declare_dram_parameter(wname, [W, H], F32, isOutput=False)
        b_e[bname] = nc.declare_dram_parameter(bname, [H], F32, isOutput=False)
    out_e = nc.declare_dram_parameter("out", [B_PER_CORE, H + 1, S], F32, isOutput=True)

    with tile.TileContext(nc, pool_alloc_mode="queue") as tc:
        _build(nc, tc, inp_e, mask_e, w_e, b_e, out_e)
    nc.finalize()
    return nc


@functools.lru_cache(maxsize=1)
def _get_nc():
    return build_nc()


def run(inputs, trace=False, **kwargs):
    nc = _get_nc()
    inp = np.ascontiguousarray(np.asarray(inputs["input"], dtype=np.float32))
    msk = np.ascontiguousarray(np.asarray(inputs["mask"], dtype=np.int32))
    shared = {
        k: np.ascontiguousarray(np.asarray(inputs[k], dtype=np.float32))
        for k in ("Wq", "bq", "Wk", "bk", "Wv", "bv")
    }
    in_maps = []
    for c in range(N_CORES):
        m = {
            "input": inp[B_PER_CORE * c : B_PER_CORE * (c + 1)],
            "mask": msk[B_PER_CORE * c : B_PER_CORE * (c + 1)],
        }
        m.update(shared)
        in_maps.append(m)
    res = run_bass_kernel_spmd(nc, in_maps, list(range(N_CORES)), trace=trace, **kwargs)
    outs = np.concatenate(
        [res.results[i]["out"] for i in range(N_CORES)], axis=0
    )  # [16, 65, 2048]
    o = outs[:, :H, :] / outs[:, H : H + 1, :]
    return np.ascontiguousarray(o.transpose(0, 2, 1)).astype(np.float32), res


def kernel(**inputs):
    out, _ = run(inputs, trace=False)
    return out



# revision 5
# speedup vs baseline: 1.2216x; 1.2216x over previous
"""Trainium2 Bass kernel for a single attention head (nn_AttentionHead).

Problem: B=16, S=2048, W=768, H=64.
  Q = input @ Wq + bq ; K = input @ Wk + bk ; V = input @ Wv + bv
  scores = Q K^T / sqrt(H), key-padding mask, softmax, out = attn @ V.

Sharding: data-parallel over batch across 8 cores (2 samples per core).

v3 design (per core). Two cost facts drive it: TensorE matmul time
depends only on moving columns (contraction depth is free), and ScalarE
exp costs 0.83 ns per score-matrix column. Both scale with the number of
KEY tiles, and masked keys (about half: exp == 0 exactly) contribute
nothing — so the host compacts each sample's keys to the valid subset
(padded to a whole, even number of 128-key tiles; pad keys get a -100
exp bias so they are exactly zero, making compaction bit-equivalent).

  1. Host packs X^T bf16 [B, P, NW, S] for the Q pass, the compacted
     X_kv^T bf16 [B, P, NW, SKV] for the K/V pass, stationaries
     Wq / [Wk|Wv], biases, and the exp bias table (layout prep only).
  2. Q projection (bf16, moving X^T) -> Q^T [64, S]; K/V projection
     (bf16, packed stationary, moving X_kv^T) -> kv [K^T rows 0:64 |
     V^T rows 64:128] over SKV compacted keys. DVE bias-add evacuations.
  3. Scores transposed S^T[k, q] = K^T.T Q^T, plain bf16 matmuls with
     64-deep contraction (cost is moving columns, so depth 64 is free).
  4. exp on ScalarE out of PSUM, scale=1/8 (absorbs 1/sqrt(H); weights
     stay unscaled), bias = -2 margin or -102 for pad keys; the margin
     cancels in the final divide.
  5. V' = [V | ones] rebuilt natural per key tile by TensorE transposes
     of kv rows 64:128 (identity corner at base partition 64). O'^T
     [65, S] accumulated over compacted key tiles in PSUM; row 64 is the
     softmax denominator.
  6. Sample 1's entire prologue is interleaved into sample 0's attention
     loop so TensorE/DVE/DMA work overlaps the exp stream.
  7. Host epilogue: O = O'[:64] / O'[64], transpose to [B, S, H].
"""

import functools

import ml_dtypes
import numpy as np

import concourse.bass as bass
import concourse.bacc as bacc
import concourse.mybir as mybir
import concourse.tile as tile
from concourse.bass_utils import run_bass_kernel_spmd
from concourse.masks import make_identity

F32 = mybir.dt.float32
BF16 = mybir.dt.bfloat16
AF = mybir.ActivationFunctionType
ALU = mybir.AluOpType

P = 128
B_PER_CORE = 2
S = 2048
W = 768
H = 64
NW = W // P      # 6 contraction chunks for the projections
NKT = S // P     # 16 key tiles uncompacted
NQC = S // 512   # 4 query chunks of 512
N_CORES = 8
PAD_BIAS = -100.0   # exp bias for pad keys (exp -> 0 exactly in bf16)
EXP_MARGIN = -2.0   # global exp bias margin (cancels in the divide)
QSCALE = 0.125      # 1/sqrt(H), applied as the exp scale

NP_BF16 = ml_dtypes.bfloat16


def _kv_chunks(skv):
    """PSUM-bank-sized (<=512 col) chunks covering the compacted keys."""
    edges = list(range(0, skv, 512)) + [skv]
    return list(zip(edges[:-1], edges[1:]))


def _emit_q_proj(nc, pools, b, qc):
    wq, bq, xt, qt, wps = (
        pools["wq"], pools["bq"], pools["xt"][b], pools["qt"][b], pools["wps"],
    )
    ps = wps.tile([P, 1024], F32, tag="wps", name=f"pq_{b}_{qc}")
    for wc in range(NW):
        nc.tensor.matmul(
            ps[0:H, 0:512],
            wq[:, wc, :],
            xt[:, wc, qc * 512 : (qc + 1) * 512],
            start=(wc == 0),
            stop=(wc == NW - 1),
        )
    nc.vector.tensor_scalar(
        qt[:, qc * 512 : (qc + 1) * 512], ps[0:H, 0:512], bq, None, ALU.add
    )


def _emit_kv_proj(nc, pools, b, c0, c1):
    wkv, bkv, xkv, kv, wps = (
        pools["wkv"], pools["bkv"], pools["xkv"][b], pools["kv"][b], pools["wps"],
    )
    ps = wps.tile([P, 1024], F32, tag="wps", name=f"pkv_{b}_{c0}")
    for wc in range(NW):
        nc.tensor.matmul(
            ps[:, 0 : c1 - c0],
            wkv[:, wc, :],
            xkv[:, wc, c0:c1],
            start=(wc == 0),
            stop=(wc == NW - 1),
        )
    nc.vector.tensor_scalar(kv[:, c0:c1], ps[:, 0 : c1 - c0], bkv, None, ALU.add)


def _emit_vtrans(nc, pools, b, j):
    """Transpose kv rows 64:128 (V^T) for key-tile pair (2j, 2j+1) into
    natural bf16 V' tiles."""
    kv, vp, ident, wps = (
        pools["kv"][b], pools["vp"][b], pools["ident"], pools["wps"],
    )
    pst = wps.tile([P, P], BF16, tag="wps", name=f"pvt_{b}_{j}")
    for i in range(2):
        kt = 2 * j + i
        nc.tensor.transpose(
            pst[:, i * H : (i + 1) * H],
            kv[H:P, kt * P : (kt + 1) * P],
            ident[H:P, H:P],
        )
    nc.vector.tensor_copy(
        vp[:, 2 * j : 2 * j + 2, 0:H], pst.rearrange("p (i h) -> p i h", h=H)
    )


def _prologue_stages(nc, pools, b, skv):
    stages = []
    for qc in range(NQC):
        stages.append(functools.partial(_emit_q_proj, nc, pools, b, qc))
    for c0, c1 in _kv_chunks(skv):
        stages.append(functools.partial(_emit_kv_proj, nc, pools, b, c0, c1))
    for j in range(skv // 256):
        stages.append(functools.partial(_emit_vtrans, nc, pools, b, j))
    return stages


def _emit_attention(nc, pools, b, out_e, nkt_kv, interleave=()):
    """Score -> exp -> PV loop for sample b over the compacted key tiles.
    interleave[kt] is a list of thunks emitted at the top of iteration kt
    (the other sample's prologue, to fill engine gaps under the exp
    stream)."""
    qt, kv, vp, ebias = (
        pools["qt"][b], pools["kv"][b], pools["vp"][b], pools["ebias"][b],
    )
    wps, ptp, pso_p, oup = pools["wps"], pools["ptp"], pools["pso"], pools["oup"]

    # ones column of V' (row 64 of O'^T = softmax denominator)
    nc.gpsimd.memset(vp[:, :, H : H + 1], 1.0)

    pso = pso_p.tile([H + 1, S], F32, tag="pso", name=f"pso{b}")
    for kt in range(nkt_kv):
        for thunk in (interleave[kt] if kt < len(interleave) else ()):
            thunk()
        pt = ptp.tile([P, S], BF16, tag="pt", name=f"pt_{b}_{kt}")
        for qh in range(2):
            sps = wps.tile([P, 1024], F32, tag="wps", name=f"ss_{b}_{kt}_{qh}")
            for qi in range(2):
                qlo = qh * 1024 + qi * 512
                nc.tensor.matmul(
                    sps[:, qi * 512 : (qi + 1) * 512],
                    kv[0:H, kt * P : (kt + 1) * P],
                    qt[:, qlo : qlo + 512],
                    start=True,
                    stop=True,
                )
            nc.scalar.activation(
                pt[:, qh * 1024 : (qh + 1) * 1024],
                sps,
                AF.Exp,
                bias=ebias[:, kt : kt + 1],
                scale=QSCALE,
            )
        for qc in range(NQC):
            nc.tensor.matmul(
                pso[:, qc * 512 : (qc + 1) * 512],
                vp[:, kt, :],
                pt[:, qc * 512 : (qc + 1) * 512],
                start=(kt == 0),
                stop=(kt == nkt_kv - 1),
            )
    ou = oup.tile([H + 1, S], F32, tag="ou", name=f"ou{b}")
    nc.vector.tensor_copy(ou, pso)
    nc.sync.dma_start(out=out_e[b], in_=ou)


def _build(nc, tc, nkt_kv, xt_e, xkv_e, eb_e, wq_e, wkv_e, bq_e, bkv_e, out_e):
    skv = nkt_kv * P
    with (
        tc.tile_pool(name="const", bufs=1) as cpool,
        tc.tile_pool(name="xtp", bufs=2) as xtp,
        tc.tile_pool(name="xkvp", bufs=2) as xkvp,
        tc.tile_pool(name="qtp", bufs=2) as qtp,
        tc.tile_pool(name="kvp", bufs=2) as kvp,
        tc.tile_pool(name="vpp", bufs=2) as vpp,
        tc.tile_pool(name="ptp", bufs=2) as ptp,
        tc.tile_pool(name="oup", bufs=2) as oup,
        tc.tile_pool(name="ebp", bufs=2) as ebp,
        tc.tile_pool(name="wps", bufs=2, space="PSUM") as wps,
        tc.tile_pool(name="psop", bufs=1, space="PSUM") as pso_p,
    ):
        ident = cpool.tile([P, P], BF16, name="ident", tag="ident")
        make_identity(nc, ident)
        wq = cpool.tile([P, NW, H], BF16, name="wq", tag="wq")
        wkv = cpool.tile([P, NW, P], BF16, name="wkv", tag="wkv")
        bq = cpool.tile([H, 1], F32, name="bq", tag="bq")
        bkv = cpool.tile([P, 1], F32, name="bkv", tag="bkv")
        nc.gpsimd.dma_start(out=wq, in_=wq_e[:, :, :])
        nc.gpsimd.dma_start(out=wkv, in_=wkv_e[:, :, :])
        nc.gpsimd.dma_start(out=bq, in_=bq_e[:, :])
        nc.gpsimd.dma_start(out=bkv, in_=bkv_e[:, :])

        pools = {
            "ident": ident, "wq": wq, "wkv": wkv, "bq": bq, "bkv": bkv,
            "wps": wps, "pso": pso_p, "ptp": ptp, "oup": oup,
            "xt": [], "xkv": [], "qt": [], "kv": [], "vp": [], "ebias": [],
        }
        for b in range(B_PER_CORE):
            eb = ebp.tile([P, nkt_kv], F32, tag="eb", name=f"eb{b}")
            nc.gpsimd.dma_start(out=eb, in_=eb_e[b])
            pools["ebias"].append(eb)
            pools["xt"].append(xtp.tile([P, NW, S], BF16, tag="xt", name=f"xt{b}"))
            pools["xkv"].append(
                xkvp.tile([P, NW, skv], BF16, tag="xkv", name=f"xkv{b}")
            )
            pools["qt"].append(qtp.tile([H, S], BF16, tag="qt", name=f"qt{b}"))
            pools["kv"].append(kvp.tile([P, skv], BF16, tag="kv", name=f"kv{b}"))
            pools["vp"].append(
                vpp.tile([P, nkt_kv, H + 1], BF16, tag="vp", name=f"vp{b}")
            )

        # input loads, sliced so the first projection groups start early;
        # sample 0 first.
        for b in range(B_PER_CORE):
            for qc in range(NQC):
                for wc in range(NW):
                    nc.sync.dma_start(
                        out=pools["xt"][b][:, wc, qc * 512 : (qc + 1) * 512],
                        in_=xt_e[b, :, wc, qc * 512 : (qc + 1) * 512],
                    )
            for c0, c1 in _kv_chunks(skv):
                for wc in range(NW):
                    nc.sync.dma_start(
                        out=pools["xkv"][b][:, wc, c0:c1],
                        in_=xkv_e[b, :, wc, c0:c1],
                    )

        # Sample 0: Q projection, KV chunk 0 and vtrans 0 up front; the
        # rest of its V-side work interleaves into the first attention
        # iterations (PV for tile kt only needs V' tile kt).
        s0 = _prologue_stages(nc, pools, 0, skv)
        s1 = _prologue_stages(nc, pools, 1, skv)
        nch = len(_kv_chunks(skv))
        npair = skv // 256
        for st in s0[: NQC + 1]:  # Q proj x4, KV chunk 0
            st()
        s0[NQC + nch]()  # vtrans 0
        inter = [[] for _ in range(nkt_kv)]
        for c in range(1, nch):  # KV chunk c at kt=c-1 (covers kts 4c..)
            inter[c - 1].append(s0[NQC + c])
        for j in range(1, npair):  # vtrans j at kt=2j-1 (consumed at 2j)
            inter[2 * j - 1].append(s0[NQC + nch + j])
        # sample 1's full prologue spread over kt=2..nkt_kv-1
        lo = min(2, nkt_kv - 1)
        span = max(nkt_kv - lo, 1)
        for i, st in enumerate(s1):
            inter[lo + (i * span) // len(s1)].append(st)
        _emit_attention(nc, pools, 0, out_e, nkt_kv, interleave=inter)
        _emit_attention(nc, pools, 1, out_e, nkt_kv)


@functools.lru_cache(maxsize=2)
def build_nc(nkt_kv: int) -> bass.Bass:
    skv = nkt_kv * P
    nc = bacc.Bacc()
    xt_e = nc.declare_dram_parameter("xt", [B_PER_CORE, P, NW, S], BF16, isOutput=False)
    xkv_e = nc.declare_dram_parameter(
        "xkv", [B_PER_CORE, P, NW, skv], BF16, isOutput=False
    )
    eb_e = nc.declare_dram_parameter("eb", [B_PER_CORE, P, nkt_kv], F32, isOutput=False)
    wq_e = nc.declare_dram_parameter("wq", [P, NW, H], BF16, isOutput=False)
    wkv_e = nc.declare_dram_parameter("wkv", [P, NW, P], BF16, isOutput=False)
    bq_e = nc.declare_dram_parameter("bq", [H, 1], F32, isOutput=False)
    bkv_e = nc.declare_dram_parameter("bkv", [P, 1], F32, isOutput=False)
    out_e = nc.declare_dram_parameter("out", [B_PER_CORE, H + 1, S], F32, isOutput=True)

    with tile.TileContext(nc, pool_alloc_mode="queue") as tc:
        _build(nc, tc, nkt_kv, xt_e, xkv_e, eb_e, wq_e, wkv_e, bq_e, bkv_e, out_e)
    nc.finalize()
    return nc


def _host_prep(inputs):
    """Pack the full inputs into per-core DRAM layouts (layout/dtype/
    gather prep only; all arithmetic stays on device)."""
    inp = np.asarray(inputs["input"], dtype=np.float32)      # [16, S, W]
    msk = np.asarray(inputs["mask"], dtype=np.int32)         # [16, 1, S]
    B = inp.shape[0]

    # X^T[b, p, wc, s] = X[b, s, wc*128 + p]
    def pack_t(x):
        s = x.shape[1]
        return np.ascontiguousarray(
            x.transpose(0, 2, 1).reshape(B, NW, P, s).transpose(0, 2, 1, 3)
        ).astype(NP_BF16)

    xt = pack_t(inp)

    # compact the keys: per sample gather the valid positions, pad to an
    # even number of whole 128-key tiles (shared across cores: SPMD)
    valid = [np.nonzero(msk[b, 0])[0] for b in range(B)]
    nv_max = max(len(v) for v in valid)
    nkt_kv = min(-(-nv_max // P), NKT)
    nkt_kv = min(nkt_kv + (nkt_kv % 2), NKT)
    skv = nkt_kv * P

    xkv_rows = np.zeros((B, skv, W), dtype=np.float32)
    eb = np.full((B, skv), PAD_BIAS, dtype=np.float32)
    for b in range(B):
        v = valid[b][:skv]
        xkv_rows[b, : len(v)] = inp[b, v]
        eb[b, : len(v)] = 0.0
    xkv = pack_t(xkv_rows)
    eb = (eb + EXP_MARGIN).reshape(B, nkt_kv, P).transpose(0, 2, 1)
    eb = np.ascontiguousarray(eb)

    wq_in = np.asarray(inputs["Wq"], dtype=np.float32)
    wk = np.asarray(inputs["Wk"], dtype=np.float32)
    wv = np.asarray(inputs["Wv"], dtype=np.float32)
    wq = np.ascontiguousarray(wq_in.reshape(NW, P, H).transpose(1, 0, 2)).astype(
        NP_BF16
    )
    wkv = np.concatenate([wk, wv], axis=1).reshape(NW, P, 2 * H).transpose(1, 0, 2)
    wkv = np.ascontiguousarray(wkv).astype(NP_BF16)

    bq = np.asarray(inputs["bq"], dtype=np.float32)[:, None]
    bkv = np.concatenate(
        [np.asarray(inputs["bk"]), np.asarray(inputs["bv"])]
    ).astype(np.float32)[:, None]
    return nkt_kv, xt, xkv, eb, wq, wkv, bq, bkv


def run(inputs, trace=False, **kwargs):
    nkt_kv, xt, xkv, eb, wq, wkv, bq, bkv = _host_prep(inputs)
    nc = build_nc(nkt_kv)
    in_maps = []
    for c in range(N_CORES):
        sl = slice(B_PER_CORE * c, B_PER_CORE * (c + 1))
        in_maps.append({
            "xt": xt[sl], "xkv": xkv[sl], "eb": eb[sl],
            "wq": wq, "wkv": wkv, "bq": bq, "bkv": bkv,
        })
    res = run_bass_kernel_spmd(nc, in_maps, list(range(N_CORES)), trace=trace, **kwargs)
    outs = np.concatenate(
        [res.results[i]["out"] for i in range(N_CORES)], axis=0
    )  # [16, 65, 2048]
    o = outs[:, :H, :] / outs[:, H : H + 1, :]
    return np.ascontiguousarray(o.transpose(0, 2, 1)).astype(np.float32), res


def kernel(**inputs):
    out, _ = run(inputs, trace=False)
    return out


# revision 6
# speedup vs baseline: 1.5518x; 1.2703x over previous
"""Trainium2 Bass kernel for a single attention head (nn_AttentionHead).

Problem: B=16, S=2048, W=768, H=64.
  Q = input @ Wq + bq ; K = input @ Wk + bk ; V = input @ Wv + bv
  scores = Q K^T / sqrt(H), key-padding mask, softmax, out = attn @ V.

Sharding: data-parallel over batch across 8 cores (2 samples per core).

v3 design (per core). Two cost facts drive it: TensorE matmul time
depends only on moving columns (contraction depth is free), and ScalarE
exp costs 0.83 ns per score-matrix column. Both scale with the number of
KEY tiles, and masked keys (about half: exp == 0 exactly) contribute
nothing — so the host compacts each sample's keys to the valid subset
(padded to a whole, even number of 128-key tiles; pad keys get a -100
exp bias so they are exactly zero, making compaction bit-equivalent).

  1. Host packs X^T bf16 [B, P, NW, S] for the Q pass, the compacted
     X_kv^T bf16 [B, P, NW, SKV] for the K/V pass, stationaries
     Wq / [Wk|Wv], biases, and the exp bias table (layout prep only).
  2. Q projection (bf16, moving X^T) -> Q^T [64, S]; K/V projection
     (bf16, packed stationary, moving X_kv^T) -> kv [K^T rows 0:64 |
     V^T rows 64:128] over SKV compacted keys. DVE bias-add evacuations.
  3. Scores transposed S^T[k, q] = K^T.T Q^T, plain bf16 matmuls with
     64-deep contraction (cost is moving columns, so depth 64 is free).
  4. exp on ScalarE out of PSUM, scale=1/8 (absorbs 1/sqrt(H); weights
     stay unscaled), bias = -2 margin or -102 for pad keys; the margin
     cancels in the final divide.
  5. V' = [V | ones] rebuilt natural per key tile by TensorE transposes
     of kv rows 64:128 (identity corner at base partition 64). O'^T
     [65, S] accumulated over compacted key tiles in PSUM; row 64 is the
     softmax denominator.
  6. Sample 1's entire prologue is interleaved into sample 0's attention
     loop so TensorE/DVE/DMA work overlaps the exp stream.
  7. Host epilogue: O = O'[:64] / O'[64], transpose to [B, S, H].
"""

import functools

import ml_dtypes
import numpy as np

import concourse.bass as bass
import concourse.bacc as bacc
import concourse.mybir as mybir
import concourse.tile as tile
from concourse.bass_utils import run_bass_kernel_spmd
from concourse.masks import make_identity

F32 = mybir.dt.float32
BF16 = mybir.dt.bfloat16
AF = mybir.ActivationFunctionType
ALU = mybir.AluOpType

P = 128
B_PER_CORE = 2
S = 2048
W = 768
H = 64
NW = W // P      # 6 contraction chunks for the projections
NKT = S // P     # 16 key tiles uncompacted
NQC = S // 512   # 4 query chunks of 512
N_CORES = 8
PAD_BIAS = -100.0   # exp bias for pad keys (exp -> 0 exactly in bf16)
EXP_MARGIN = -2.0   # global exp bias margin (cancels in the divide)
QSCALE = 0.125      # 1/sqrt(H), applied as the exp scale

NP_BF16 = ml_dtypes.bfloat16


def _kv_chunks(skv):
    """PSUM-bank-sized (<=512 col) chunks covering the compacted keys."""
    edges = list(range(0, skv, 512)) + [skv]
    return list(zip(edges[:-1], edges[1:]))


def _emit_q_proj(nc, pools, b, qc):
    wq, bq, xt, qt, pps = (
        pools["wq"], pools["bq"], pools["xt"][b], pools["qt"][b], pools["pps"],
    )
    ps = pps.tile([P, 512], F32, tag="pps", name=f"pq_{b}_{qc}")
    for wc in range(NW):
        nc.tensor.matmul(
            ps[0:H, :],
            wq[:, wc, :],
            xt[:, wc, qc * 512 : (qc + 1) * 512],
            start=(wc == 0),
            stop=(wc == NW - 1),
        )
    nc.vector.tensor_scalar(
        qt[:, qc * 512 : (qc + 1) * 512], ps[0:H, :], bq, None, ALU.add
    )


def _emit_kv_proj(nc, pools, b, c0, c1):
    wkv, bkv, xkv, kv, pps = (
        pools["wkv"], pools["bkv"], pools["xkv"][b], pools["kv"][b], pools["pps"],
    )
    ps = pps.tile([P, 512], F32, tag="pps", name=f"pkv_{b}_{c0}")
    for wc in range(NW):
        nc.tensor.matmul(
            ps[:, 0 : c1 - c0],
            wkv[:, wc, :],
            xkv[:, wc, c0:c1],
            start=(wc == 0),
            stop=(wc == NW - 1),
        )
    nc.vector.tensor_scalar(kv[:, c0:c1], ps[:, 0 : c1 - c0], bkv, None, ALU.add)


def _emit_vtrans(nc, pools, b, j):
    """Transpose kv rows 64:128 (V^T) for key-tile pair (2j, 2j+1) into
    natural bf16 V' tiles."""
    kv, vp, ident, sps = (
        pools["kv"][b], pools["vp"][b], pools["ident"], pools["sps"],
    )
    pst = sps.tile([P, P], BF16, tag="sps", name=f"pvt_{b}_{j}")
    for i in range(2):
        kt = 2 * j + i
        nc.tensor.transpose(
            pst[:, i * H : (i + 1) * H],
            kv[H:P, kt * P : (kt + 1) * P],
            ident[H:P, H:P],
        )
    nc.vector.tensor_copy(
        vp[:, 2 * j : 2 * j + 2, 0:H], pst.rearrange("p (i h) -> p i h", h=H)
    )


def _prologue_stages(nc, pools, b, skv):
    stages = []
    for qc in range(NQC):
        stages.append(functools.partial(_emit_q_proj, nc, pools, b, qc))
    for c0, c1 in _kv_chunks(skv):
        stages.append(functools.partial(_emit_kv_proj, nc, pools, b, c0, c1))
    for j in range(skv // 256):
        stages.append(functools.partial(_emit_vtrans, nc, pools, b, j))
    return stages


def _emit_attention(nc, pools, b, out_e, nkt_kv, interleave=()):
    """Score -> exp -> PV loop for sample b over the compacted key tiles.
    interleave[kt] is a list of thunks emitted at the top of iteration kt
    (the other sample's prologue, to fill engine gaps under the exp
    stream)."""
    qt, kv, vp, ebias = (
        pools["qt"][b], pools["kv"][b], pools["vp"][b], pools["ebias"][b],
    )
    sps_p, ptp, pso_p, oup = pools["sps"], pools["ptp"], pools["pso"], pools["oup"]

    # ones column of V' (row 64 of O'^T = softmax denominator)
    nc.gpsimd.memset(vp[:, :, H : H + 1], 1.0)

    pso = pso_p.tile([H + 1, S], F32, tag="pso", name=f"pso{b}")
    for kt in range(nkt_kv):
        for thunk in (interleave[kt] if kt < len(interleave) else ()):
            thunk()
        pt = ptp.tile([P, S], BF16, tag="pt", name=f"pt_{b}_{kt}")
        for qc in range(NQC):
            sps = sps_p.tile([P, 512], F32, tag="sps", name=f"ss_{b}_{kt}_{qc}")
            nc.tensor.matmul(
                sps,
                kv[0:H, kt * P : (kt + 1) * P],
                qt[:, qc * 512 : (qc + 1) * 512],
                start=True,
                stop=True,
            )
            nc.scalar.activation(
                pt[:, qc * 512 : (qc + 1) * 512],
                sps,
                AF.Exp,
                bias=ebias[:, kt : kt + 1],
                scale=QSCALE,
            )
        for qc in range(NQC):
            nc.tensor.matmul(
                pso[:, qc * 512 : (qc + 1) * 512],
                vp[:, kt, :],
                pt[:, qc * 512 : (qc + 1) * 512],
                start=(kt == 0),
                stop=(kt == nkt_kv - 1),
            )
    ou = oup.tile([H + 1, S], F32, tag="ou", name=f"ou{b}")
    for half in range(2):
        sl = slice(half * (S // 2), (half + 1) * (S // 2))
        nc.vector.tensor_copy(ou[:, sl], pso[:, sl])
        nc.sync.dma_start(out=out_e[b, :, sl], in_=ou[:, sl])


def _build(nc, tc, nkt_kv, xt_e, xkv_e, eb_e, wq_e, wkv_e, bq_e, bkv_e, out_e):
    skv = nkt_kv * P
    with (
        tc.tile_pool(name="const", bufs=1) as cpool,
        tc.tile_pool(name="xtp", bufs=2) as xtp,
        tc.tile_pool(name="xkvp", bufs=2) as xkvp,
        tc.tile_pool(name="qtp", bufs=2) as qtp,
        tc.tile_pool(name="kvp", bufs=2) as kvp,
        tc.tile_pool(name="vpp", bufs=2) as vpp,
        tc.tile_pool(name="ptp", bufs=2) as ptp,
        tc.tile_pool(name="oup", bufs=2) as oup,
        tc.tile_pool(name="ebp", bufs=2) as ebp,
        tc.tile_pool(name="sps", bufs=2, space="PSUM") as sps_p,
        tc.tile_pool(name="pps", bufs=2, space="PSUM") as pps,
        tc.tile_pool(name="psop", bufs=1, space="PSUM") as pso_p,
    ):
        ident = cpool.tile([P, P], BF16, name="ident", tag="ident")
        make_identity(nc, ident)
        wq = cpool.tile([P, NW, H], BF16, name="wq", tag="wq")
        wkv = cpool.tile([P, NW, P], BF16, name="wkv", tag="wkv")
        bq = cpool.tile([H, 1], F32, name="bq", tag="bq")
        bkv = cpool.tile([P, 1], F32, name="bkv", tag="bkv")
        nc.gpsimd.dma_start(out=wq, in_=wq_e[:, :, :])
        nc.gpsimd.dma_start(out=wkv, in_=wkv_e[:, :, :])
        nc.gpsimd.dma_start(out=bq, in_=bq_e[:, :])
        nc.gpsimd.dma_start(out=bkv, in_=bkv_e[:, :])

        pools = {
            "ident": ident, "wq": wq, "wkv": wkv, "bq": bq, "bkv": bkv,
            "sps": sps_p, "pps": pps, "pso": pso_p, "ptp": ptp, "oup": oup,
            "xt": [], "xkv": [], "qt": [], "kv": [], "vp": [], "ebias": [],
        }
        for b in range(B_PER_CORE):
            eb = ebp.tile([P, nkt_kv], F32, tag="eb", name=f"eb{b}")
            nc.gpsimd.dma_start(out=eb, in_=eb_e[b])
            pools["ebias"].append(eb)
            pools["xt"].append(xtp.tile([P, NW, S], BF16, tag="xt", name=f"xt{b}"))
            pools["xkv"].append(
                xkvp.tile([P, NW, skv], BF16, tag="xkv", name=f"xkv{b}")
            )
            pools["qt"].append(qtp.tile([H, S], BF16, tag="qt", name=f"qt{b}"))
            pools["kv"].append(kvp.tile([P, skv], BF16, tag="kv", name=f"kv{b}"))
            pools["vp"].append(
                vpp.tile([P, nkt_kv, H + 1], BF16, tag="vp", name=f"vp{b}")
            )

        # input loads, sliced so the first projection groups start early;
        # sample 0 first.
        chunks = _kv_chunks(skv)
        for b in range(B_PER_CORE):
            # first KV chunk before the X^T bulk: the first score matmuls
            # need kv chunk 0 while Q projection is still streaming
            plan = [("xkv", chunks[0])] + [
                ("xt", (qc * 512, (qc + 1) * 512)) for qc in range(NQC)
            ] + [("xkv", c) for c in chunks[1:]]
            for kind, (c0, c1) in plan:
                dst = pools[kind][b]
                src_e = xt_e if kind == "xt" else xkv_e
                for wc in range(NW):
                    nc.sync.dma_start(
                        out=dst[:, wc, c0:c1],
                        in_=src_e[b, :, wc, c0:c1],
                    )

        # Sample 0: Q projection, KV chunk 0 and vtrans 0 up front; the
        # rest of its V-side work interleaves into the first attention
        # iterations (PV for tile kt only needs V' tile kt).
        s0 = _prologue_stages(nc, pools, 0, skv)
        s1 = _prologue_stages(nc, pools, 1, skv)
        nch = len(_kv_chunks(skv))
        npair = skv // 256
        s0[NQC]()  # KV chunk 0 first: scores(kt 0..3) need it
        for st in s0[:NQC]:  # Q proj x4
            st()
        s0[NQC + nch]()  # vtrans 0
        inter = [[] for _ in range(nkt_kv)]
        for c in range(1, nch):  # KV chunk c at kt=c-1 (covers kts 4c..)
            inter[c - 1].append(s0[NQC + c])
        for j in range(1, npair):  # vtrans j at kt=2j-1 (consumed at 2j)
            inter[2 * j - 1].append(s0[NQC + nch + j])
        # sample 1's full prologue spread over kt=2..nkt_kv-1
        lo = min(2, nkt_kv - 1)
        span = max(nkt_kv - lo, 1)
        for i, st in enumerate(s1):
            inter[lo + (i * span) // len(s1)].append(st)
        _emit_attention(nc, pools, 0, out_e, nkt_kv, interleave=inter)
        _emit_attention(nc, pools, 1, out_e, nkt_kv)


@functools.lru_cache(maxsize=2)
def build_nc(nkt_kv: int) -> bass.Bass:
    skv = nkt_kv * P
    nc = bacc.Bacc()
    xt_e = nc.declare_dram_parameter("xt", [B_PER_CORE, P, NW, S], BF16, isOutput=False)
    xkv_e = nc.declare_dram_parameter(
        "xkv", [B_PER_CORE, P, NW, skv], BF16, isOutput=False
    )
    eb_e = nc.declare_dram_parameter("eb", [B_PER_CORE, P, nkt_kv], F32, isOutput=False)
    wq_e = nc.declare_dram_parameter("wq", [P, NW, H], BF16, isOutput=False)
    wkv_e = nc.declare_dram_parameter("wkv", [P, NW, P], BF16, isOutput=False)
    bq_e = nc.declare_dram_parameter("bq", [H, 1], F32, isOutput=False)
    bkv_e = nc.declare_dram_parameter("bkv", [P, 1], F32, isOutput=False)
    out_e = nc.declare_dram_parameter("out", [B_PER_CORE, H + 1, S], F32, isOutput=True)

    with tile.TileContext(nc, pool_alloc_mode="queue") as tc:
        _build(nc, tc, nkt_kv, xt_e, xkv_e, eb_e, wq_e, wkv_e, bq_e, bkv_e, out_e)
    nc.finalize()
    return nc


def _host_prep(inputs):
    """Pack the full inputs into per-core DRAM layouts (layout/dtype/
    gather prep only; all arithmetic stays on device)."""
    inp = np.asarray(inputs["input"], dtype=np.float32)      # [16, S, W]
    msk = np.asarray(inputs["mask"], dtype=np.int32)         # [16, 1, S]
    B = inp.shape[0]

    # X^T[b, p, wc, s] = X[b, s, wc*128 + p]
    def pack_t(x):
        s = x.shape[1]
        return np.ascontiguousarray(
            x.transpose(0, 2, 1).reshape(B, NW, P, s).transpose(0, 2, 1, 3)
        ).astype(NP_BF16)

    xt = pack_t(inp)

    # compact the keys: per sample gather the valid positions, pad to an
    # even number of whole 128-key tiles (shared across cores: SPMD)
    valid = [np.nonzero(msk[b, 0])[0] for b in range(B)]
    nv_max = max(len(v) for v in valid)
    nkt_kv = min(-(-nv_max // P), NKT)
    nkt_kv = min(nkt_kv + (nkt_kv % 2), NKT)
    skv = nkt_kv * P

    xkv_rows = np.zeros((B, skv, W), dtype=np.float32)
    eb = np.full((B, skv), PAD_BIAS, dtype=np.float32)
    for b in range(B):
        v = valid[b][:skv]
        xkv_rows[b, : len(v)] = inp[b, v]
        eb[b, : len(v)] = 0.0
    xkv = pack_t(xkv_rows)
    eb = (eb + EXP_MARGIN).reshape(B, nkt_kv, P).transpose(0, 2, 1)
    eb = np.ascontiguousarray(eb)

    wq_in = np.asarray(inputs["Wq"], dtype=np.float32)
    wk = np.asarray(inputs["Wk"], dtype=np.float32)
    wv = np.asarray(inputs["Wv"], dtype=np.float32)
    wq = np.ascontiguousarray(wq_in.reshape(NW, P, H).transpose(1, 0, 2)).astype(
        NP_BF16
    )
    wkv = np.concatenate([wk, wv], axis=1).reshape(NW, P, 2 * H).transpose(1, 0, 2)
    wkv = np.ascontiguousarray(wkv).astype(NP_BF16)

    bq = np.asarray(inputs["bq"], dtype=np.float32)[:, None]
    bkv = np.concatenate(
        [np.asarray(inputs["bk"]), np.asarray(inputs["bv"])]
    ).astype(np.float32)[:, None]
    return nkt_kv, xt, xkv, eb, wq, wkv, bq, bkv


def run(inputs, trace=False, **kwargs):
    nkt_kv, xt, xkv, eb, wq, wkv, bq, bkv = _host_prep(inputs)
    nc = build_nc(nkt_kv)
    in_maps = []
    for c in range(N_CORES):
        sl = slice(B_PER_CORE * c, B_PER_CORE * (c + 1))
        in_maps.append({
            "xt": xt[sl], "xkv": xkv[sl], "eb": eb[sl],
            "wq": wq, "wkv": wkv, "bq": bq, "bkv": bkv,
        })
    res = run_bass_kernel_spmd(nc, in_maps, list(range(N_CORES)), trace=trace, **kwargs)
    outs = np.concatenate(
        [res.results[i]["out"] for i in range(N_CORES)], axis=0
    )  # [16, 65, 2048]
    o = outs[:, :H, :] / outs[:, H : H + 1, :]
    return np.ascontiguousarray(o.transpose(0, 2, 1)).astype(np.float32), res


def kernel(**inputs):
    out, _ = run(inputs, trace=False)
    return out
